# revision 19
# baseline (speedup 1.0000x reference)
"""GRUCell + LayerNorm readout fused Bass kernel for Trainium2 (8 NeuronCores).

Problem: B=8192, D=H=O=1024 fp32.
    r = sigmoid(x@Wir + bir + h@Whr)
    z = sigmoid(x@Wiz + biz + h@Whz)
    n = tanh(x@Win + bin_ + r*(h@Whn + bhn))
    new_h = (1-z)*n + z*h
    out = LayerNorm(new_h)*ln_scale + ln_bias) @ Wout + bout

Strategy:
  - Data-parallel over batch: core c gets rows [c*1024, (c+1)*1024); weights
    replicated. No collectives.
  - Everything computed in the transposed domain: activations live as
    [feature, batch] so matmuls take the weights in natural [k, h] layout as
    the stationary operand and xT/hT as the moving operand, and the per-h gate
    biases become per-partition activation biases. Host passes xT/hT and
    transposes the outputs back.
  - float32r matmuls: 4x faster than fp32 on the PE at ~1.5e-4 rel error.
  - LayerNorm reduces over h = partition dim -> stats via ones-column matmuls
    on the PE. The normalize-then-matmul is algebraically folded:
        LN(new_h) @ (ln_scale*Wout) + (ln_bias@Wout + bout)
      = rstd[b]*( new_h@WoutF - mu[b]*colsum[o] ) + boutF[o]
    with WoutF = ln_scale[:,None]*Wout (host), colsum = ln_scale@Wout (host),
    boutF = bout + ln_bias@Wout (host).
"""

import sys
from contextlib import ExitStack

sys.path.insert(0, "/opt/trn_rl_repo")

import numpy as np

import concourse.bacc as bacc
import concourse.mybir as mybir
import concourse.tile as tile
from concourse import bass_utils

B, D, H, O = 8192, 1024, 1024, 1024
NCORES = 8
BL = B // NCORES          # batch rows per core
P = 128                   # partitions
KT = D // P               # contraction tiles (8)
HT = H // P               # h output-partition tiles (8)
OT = O // P               # o output-partition tiles (8)
NB = 2                    # batch chunks per core (free dim 512)
NF = BL // NB             # free dim per chunk (512)
LN_EPS = 1e-6

F32 = mybir.dt.float32
F32R = mybir.dt.float32r

_COMPILED = None  # compiled Bacc module cache across calls
TRACE = False     # set by test harness to capture an NTFF profile
LAST_RES = None   # BassKernelResults of the last run (for the test harness)


def _build():
    nc = bacc.Bacc("TRN2", target_bir_lowering=False, debug=False,
                   num_devices=NCORES)

    def din(name, shape, dt=F32R):
        return nc.dram_tensor(name, shape, dt, kind="ExternalInput").ap()

    def dout(name, shape, dt=F32R):
        return nc.dram_tensor(name, shape, dt, kind="ExternalOutput").ap()

    xT_d = din("xT", [D, BL])
    hT_d = din("hT", [H, BL])
    w_d = {g: din(f"W{g}", [D, H]) for g in ("ir", "iz", "in", "hr", "hz", "hn")}
    woutF_d = din("woutF", [H, O])
    bir_d = din("bir", [H], F32)
    biz_d = din("biz", [H], F32)
    bin_d = din("bin", [H], F32)
    bhn_d = din("bhn", [H], F32)
    boutF_d = din("boutF", [O], F32)
    colsum_d = din("colsum", [1, O])
    ones_col_d = din("ones_col", [P, 1])
    ones_row_d = din("ones_row", [1, P])

    nhT_d = dout("nhT", [H, BL], F32)
    outT_d = dout("outT", [O, BL], F32)

    with tile.TileContext(nc) as tc, ExitStack() as ctx:
        singles = ctx.enter_context(tc.tile_pool(name="singles", bufs=1))
        wpool = ctx.enter_context(tc.tile_pool(name="wpool", bufs=2))
        gates = ctx.enter_context(tc.tile_pool(name="gates", bufs=1))
        rows = ctx.enter_context(tc.tile_pool(name="rows", bufs=1))
        outp = ctx.enter_context(tc.tile_pool(name="outp", bufs=3))
        ps_gates = ctx.enter_context(
            tc.tile_pool(name="ps_gates", bufs=1, space="PSUM"))

        # ---- resident inputs -------------------------------------------------
        # Per-k tiles so the first matmuls only wait on their own slice.
        def load_kslices(ap_d, prefix):
            tiles = []
            for k in range(KT):
                t = singles.tile([P, BL], F32R, tag=f"{prefix}{k}",
                                 name=f"{prefix}{k}")
                nc.sync.dma_start(t[:], ap_d[k * P:(k + 1) * P, :])
                tiles.append(t)
            return tiles

        xT_sb = load_kslices(xT_d, "xk")
        hT_sb = load_kslices(hT_d, "hk")

        def load_vec(ap_d, n, tag):
            t = singles.tile([P, n // P], F32, tag=tag, name=tag)
            nc.sync.dma_start(t[:], ap_d.rearrange("(t p) -> p t", p=P))
            return t

        bir_sb = load_vec(bir_d, H, "bir_sb")
        biz_sb = load_vec(biz_d, H, "biz_sb")
        bin_sb = load_vec(bin_d, H, "bin_sb")
        bhn_sb = load_vec(bhn_d, H, "bhn_sb")
        boutF_sb = load_vec(boutF_d, O, "boutF_sb")
        colsum_sb = singles.tile([1, O], F32R)
        nc.sync.dma_start(colsum_sb[:], colsum_d)
        ones_col = singles.tile([P, 1], F32R)
        nc.sync.dma_start(ones_col[:], ones_col_d)
        ones_row = singles.tile([1, P], F32R)
        nc.sync.dma_start(ones_row[:], ones_row_d)
        eps_sb = singles.tile([1, 1], F32)
        nc.vector.memset(eps_sb[:], LN_EPS)

        new_hT_sb = singles.tile([P, HT, BL], F32R)

        # ---- phase 1: gates + new_h -----------------------------------------
        ps_stats_cm = tc.tile_pool(name="ps_stats", bufs=1, space="PSUM")
        ps_stats = ps_stats_cm.__enter__()
        # LN stat accumulators: sum and sum-of-squares per batch column.
        psum_s = [ps_stats.tile([1, NF], F32, tag=f"s{bc}", name=f"psum_s{bc}")
                  for bc in range(NB)]
        psum_q = [ps_stats.tile([1, NF], F32, tag=f"q{bc}", name=f"psum_q{bc}")
                  for bc in range(NB)]

        for ht in range(HT):
            hs = slice(ht * P, (ht + 1) * P)
            w_sb = {}
            for g in ("ir", "iz", "in", "hr", "hz", "hn"):
                t = wpool.tile([P, KT, P], F32R, tag=f"w{g}")
                nc.sync.dma_start(t[:], w_d[g][:, hs].rearrange(
                    "(t p) h -> p t h", p=P))
                w_sb[g] = t

            for bc in range(NB):
                bs = slice(bc * NF, (bc + 1) * NF)

                pr = ps_gates.tile([P, NF], F32, tag="r")
                pz = ps_gates.tile([P, NF], F32, tag="z")
                pgi = ps_gates.tile([P, NF], F32, tag="gi")
                pgh = ps_gates.tile([P, NF], F32, tag="gh")

                # k-major over the x-side first, then the h-side, so the PE
                # can start as soon as the first input slices land.
                for k in range(KT):
                    nc.tensor.matmul(pr[:], w_sb["ir"][:, k, :], xT_sb[k][:, bs],
                                     start=(k == 0), stop=False)
                    nc.tensor.matmul(pz[:], w_sb["iz"][:, k, :], xT_sb[k][:, bs],
                                     start=(k == 0), stop=False)
                    nc.tensor.matmul(pgi[:], w_sb["in"][:, k, :], xT_sb[k][:, bs],
                                     start=(k == 0), stop=(k == KT - 1))
                for k in range(KT):
                    nc.tensor.matmul(pr[:], w_sb["hr"][:, k, :], hT_sb[k][:, bs],
                                     start=False, stop=(k == KT - 1))
                    nc.tensor.matmul(pz[:], w_sb["hz"][:, k, :], hT_sb[k][:, bs],
                                     start=False, stop=(k == KT - 1))
                    nc.tensor.matmul(pgh[:], w_sb["hn"][:, k, :], hT_sb[k][:, bs],
                                     start=(k == 0), stop=(k == KT - 1))

                r_sb = gates.tile([P, NF], F32, tag="r_act")
                nc.scalar.activation(r_sb[:], pr[:],
                                     mybir.ActivationFunctionType.Sigmoid,
                                     bias=bir_sb[:, ht:ht + 1])
                z_sb = gates.tile([P, NF], F32, tag="z_act")
                nc.scalar.activation(z_sb[:], pz[:],
                                     mybir.ActivationFunctionType.Sigmoid,
                                     bias=biz_sb[:, ht:ht + 1])

                t_sb = gates.tile([P, NF], F32, tag="t")
                nc.vector.tensor_scalar(t_sb[:], pgh[:], bhn_sb[:, ht:ht + 1],
                                        None, mybir.AluOpType.add)
                nc.vector.tensor_mul(t_sb[:], t_sb[:], r_sb[:])
                nc.vector.tensor_add(t_sb[:], t_sb[:], pgi[:])
                n_sb = gates.tile([P, NF], F32, tag="n_act")
                nc.scalar.activation(n_sb[:], t_sb[:],
                                     mybir.ActivationFunctionType.Tanh,
                                     bias=bin_sb[:, ht:ht + 1])

                u_sb = gates.tile([P, NF], F32, tag="u")
                nc.vector.tensor_tensor(u_sb[:], hT_sb[ht][:, bs].bitcast(F32),
                                        n_sb[:], mybir.AluOpType.subtract)
                nc.vector.tensor_mul(u_sb[:], z_sb[:], u_sb[:])
                nh = new_hT_sb[:, ht, bs]
                nc.vector.tensor_add(nh, n_sb[:], u_sb[:])

                sq_sb = gates.tile([P, NF], F32R, tag="sq")
                nc.scalar.activation(sq_sb[:], nh.bitcast(F32),
                                     mybir.ActivationFunctionType.Square)

                nc.tensor.matmul(psum_s[bc][:], ones_col[:], nh,
                                 start=(ht == 0), stop=(ht == HT - 1))
                nc.tensor.matmul(psum_q[bc][:], ones_col[:], sq_sb[:],
                                 start=(ht == 0), stop=(ht == HT - 1))

                nc.sync.dma_start(nhT_d[hs, bs], nh.bitcast(F32))

        # ---- phase 2: LN stats + readout ------------------------------------
        nmu_row = []
        rstd_bc = []
        for bc in range(NB):
            nmu = rows.tile([1, NF], F32R, tag=f"nmu{bc}")
            nc.vector.tensor_scalar_mul(nmu[:], psum_s[bc][:], -1.0 / H)
            nmu_row.append(nmu)

            mu2 = rows.tile([1, NF], F32, tag="mu2")
            nc.vector.tensor_mul(mu2[:], nmu[:].bitcast(F32), nmu[:].bitcast(F32))
            var = rows.tile([1, NF], F32, tag="var")
            nc.vector.tensor_scalar_mul(var[:], psum_q[bc][:], 1.0 / H)
            nc.vector.tensor_tensor(var[:], var[:], mu2[:],
                                    mybir.AluOpType.subtract)
            nc.scalar.activation(var[:], var[:],
                                 mybir.ActivationFunctionType.Sqrt,
                                 bias=eps_sb[:])
            rrow = rows.tile([1, NF], F32R, tag=f"rstd{bc}")
            with nc.allow_low_precision(reason="f32r is fp32-width"):
                nc.vector.reciprocal(rrow[:], var[:])

            pb = ps_gates.tile([P, NF], F32, tag="r")
            nc.tensor.matmul(pb[:], ones_row[:], rrow[:], start=True, stop=True)
            rb = rows.tile([P, NF], F32, tag=f"rstd_bc{bc}")
            nc.vector.tensor_copy(rb[:], pb[:])
            rstd_bc.append(rb)

        # stats psum banks freed -> double-buffered readout psums
        ps_stats_cm.__exit__(None, None, None)
        ps_out = ctx.enter_context(
            tc.tile_pool(name="ps_out", bufs=2, space="PSUM"))

        for ot in range(OT):
            os_ = slice(ot * P, (ot + 1) * P)
            wo = wpool.tile([P, KT, P], F32R, tag="wo")
            nc.sync.dma_start(wo[:], woutF_d[:, os_].rearrange(
                "(t p) o -> p t o", p=P))
            for bc in range(NB):
                bs = slice(bc * NF, (bc + 1) * NF)
                po = ps_out.tile([P, NF], F32, tag="po")
                for k in range(HT):
                    nc.tensor.matmul(po[:], wo[:, k, :], new_hT_sb[:, k, bs],
                                     start=(k == 0), stop=False)
                # -= mu[b] * colsum[o]  (rank-1, K=1)
                nc.tensor.matmul(po[:], colsum_sb[0:1, os_], nmu_row[bc][:],
                                 start=False, stop=True)
                o_sb = outp.tile([P, NF], F32, tag="o")
                nc.vector.tensor_mul(o_sb[:], po[:], rstd_bc[bc][:])
                nc.vector.tensor_scalar(o_sb[:], o_sb[:],
                                        boutF_sb[:, ot:ot + 1], None,
                                        mybir.AluOpType.add)
                nc.sync.dma_start(outT_d[os_, bs], o_sb[:].bitcast(F32))

    nc.compile()
    return nc


def kernel(x, h, Wir, bir, Wiz, biz, Win, bin_, Whr, Whz, Whn, bhn,
           ln_scale, ln_bias, Wout, bout):
    global _COMPILED
    if _COMPILED is None:
        _COMPILED = _build()
    nc = _COMPILED

    x = np.asarray(x, np.float32)
    h = np.asarray(h, np.float32)
    xT = np.ascontiguousarray(x.T)
    hT = np.ascontiguousarray(h.T)
    woutF = np.ascontiguousarray(
        np.asarray(ln_scale, np.float32)[:, None] * np.asarray(Wout, np.float32))
    boutF = (np.asarray(bout, np.float32)
             + np.asarray(ln_bias, np.float32) @ np.asarray(Wout, np.float32))
    colsum = (np.asarray(ln_scale, np.float32)
              @ np.asarray(Wout, np.float32)).reshape(1, O)

    common = {
        "Wir": np.asarray(Wir, np.float32), "Wiz": np.asarray(Wiz, np.float32),
        "Win": np.asarray(Win, np.float32), "Whr": np.asarray(Whr, np.float32),
        "Whz": np.asarray(Whz, np.float32), "Whn": np.asarray(Whn, np.float32),
        "woutF": woutF,
        "bir": np.asarray(bir, np.float32), "biz": np.asarray(biz, np.float32),
        "bin": np.asarray(bin_, np.float32), "bhn": np.asarray(bhn, np.float32),
        "boutF": boutF.astype(np.float32), "colsum": colsum.astype(np.float32),
        "ones_col": np.ones((P, 1), np.float32),
        "ones_row": np.ones((1, P), np.float32),
    }
    in_maps = []
    for c in range(NCORES):
        bsl = slice(c * BL, (c + 1) * BL)
        in_maps.append({
            **common,
            "xT": np.ascontiguousarray(xT[:, bsl]),
            "hT": np.ascontiguousarray(hT[:, bsl]),
        })

    res = bass_utils.run_bass_kernel_spmd(nc, in_maps,
                                          core_ids=list(range(NCORES)),
                                          trace=TRACE)
    global LAST_RES
    LAST_RES = res
    new_hT = np.concatenate([res.results[c]["nhT"] for c in range(NCORES)],
                            axis=1)
    outT = np.concatenate([res.results[c]["outT"] for c in range(NCORES)],
                          axis=1)
    new_h = np.ascontiguousarray(new_hT.T)
    out = np.ascontiguousarray(outT.T)
    return new_h, out
